# revision 29
# baseline (speedup 1.0000x reference)
"""BestRQ VQ kernel for 8 TRN2 NeuronCores.

Data-parallel over batch: core b handles feats[b] (299 stacked rows).
codes = argmax_c <x_g, cb_cg>  (L2-norm of x is a positive per-row scalar and
the codebook rows are unit-norm, so normalization does not change the argmin).
quantized = codebook[codes] gathered via indirect DMA.
"""

import os
import sys

import numpy as np

sys.path.insert(0, "/opt/trn_rl_repo")

B, T, M = 8, 1200, 80
F, S = 7, 4
T2 = (T - F) // S + 1  # 299
D, G, Dg, C = 512, 8, 64, 8192
KIN = F * M  # 560
KT = 112  # KIN split into 5 tiles of 112
NKT = 5
NCORES = 8
ROW_CHUNKS = [(0, 128), (128, 128), (256, T2 - 256)]  # (start, nrows)
NCT = C // 512  # 16 c-tiles of 512

_CACHE = {}


def _build():
    import concourse.bass as bass
    import concourse.tile as tile
    from concourse import bacc, mybir
    from concourse.masks import make_identity

    f32 = mybir.dt.float32
    f32r = mybir.dt.float32r
    u32 = mybir.dt.uint32

    nc = bacc.Bacc(None, target_bir_lowering=False)

    feats = nc.declare_dram_parameter("feats", [T, M], f32, isOutput=False)
    proj = nc.declare_dram_parameter("proj", [KIN, D], f32, isOutput=False)
    cbt = nc.declare_dram_parameter("cbt", [4, 128, C], f32, isOutput=False)
    cbg = [
        nc.declare_dram_parameter(f"cbg{g}", [C, Dg], f32, isOutput=False)
        for g in range(G)
    ]
    cbd = [
        nc.declare_dram_parameter(f"cbd{g}", [C // 2, Dg], f32, isOutput=False)
        for g in range(G)
    ]
    quant = nc.declare_dram_parameter("quant", [T2, D], f32, isOutput=True)
    codes = nc.declare_dram_parameter("codes", [T2, G], u32, isOutput=True)

    with tile.TileContext(nc) as tc:
        with (
            tc.tile_pool(name="const", bufs=1) as constp,
            tc.tile_pool(name="stage1", bufs=1) as s1p,
            tc.tile_pool(name="cb", bufs=3) as cbp,
            tc.tile_pool(name="scores", bufs=3) as scp,
            tc.tile_pool(name="av", bufs=2) as avp,
            tc.tile_pool(name="outs", bufs=1) as outp,
            tc.tile_pool(name="small", bufs=4) as smp,
            tc.tile_pool(name="probe", bufs=1) as probep,
            tc.tile_pool(name="psum2", bufs=2, space="PSUM") as ps2,
        ):
            ident = constp.tile([128, 128], f32)
            make_identity(nc, ident[:])

            # ---- Stage 1: stacked features (strided DMA) + transpose ----
            stT = [s1p.tile([KT, T2], f32, tag=f"stT{k}", name=f"stT{k}") for k in range(NKT)]
            for rc, (r0, nr) in enumerate(ROW_CHUNKS):
                stacked = s1p.tile([128, KIN], f32, tag=f"stacked{rc}", name=f"stacked{rc}")
                src = bass.AP(
                    tensor=feats, offset=r0 * S * M, ap=[[S * M, nr], [1, KIN]]
                )
                nc.sync.dma_start(out=stacked[:nr, :], in_=src)
                for k in range(NKT):
                    pt = ps2.tile([KT, 128], f32, tag="ps", name="tp")
                    nc.tensor.transpose(
                        out=pt[:, :nr],
                        in_=stacked[:nr, k * KT : (k + 1) * KT],
                        identity=ident[:nr, :nr],
                    )
                    nc.vector.tensor_copy(
                        out=stT[k][:, r0 : r0 + nr], in_=pt[:, :nr]
                    )

            # ---- projection weights ----
            projsb = []
            for k in range(NKT):
                pw = s1p.tile([KT, D], f32, tag=f"proj{k}", name=f"projsb{k}")
                nc.sync.dma_start(
                    out=pw[:], in_=proj[k * KT : (k + 1) * KT, :]
                )
                projsb.append(pw)

            # ---- xsT = (stacked @ proj).T  as 4 tiles [128, T2] ----
            xsT = []
            for dc in range(4):
                px = ps2.tile([128, T2], f32, tag="ps", name="px")
                for k in range(NKT):
                    nc.tensor.matmul(
                        px[:],
                        projsb[k][:, dc * 128 : (dc + 1) * 128],
                        stT[k][:],
                        start=(k == 0),
                        stop=(k == NKT - 1),
                    )
                xt = s1p.tile([128, T2], f32, tag=f"xsT{dc}", name=f"xsT{dc}")
                nc.vector.tensor_copy(out=xt[:], in_=px[:])
                xsT.append(xt)

            xs_nat = [
                s1p.tile([128, D], f32, tag=f"xsnat{rc}", name=f"xsnat{rc}")
                for rc in range(len(ROW_CHUNKS))
            ]
            for rc, (r0, nr) in enumerate(ROW_CHUNKS):
                for dc in range(4):
                    ptn = ps2.tile([128, 128], f32, tag="ps", name="ptn")
                    nc.tensor.transpose(
                        out=ptn[:nr, :],
                        in_=xsT[dc][:, r0 : r0 + nr],
                        identity=ident[:, :],
                    )
                    nc.vector.tensor_copy(
                        out=xs_nat[rc][:nr, dc * 128 : (dc + 1) * 128],
                        in_=ptn[:nr, :],
                    )

            # ---- Stage 2: distances + argmax ----
            codes_sb = [
                outp.tile([128, G], u32, tag=f"codes{rc}", name=f"codes{rc}")
                for rc in range(len(ROW_CHUNKS))
            ]
            qsb_all = [
                outp.tile([128, D], f32, tag=f"qout{rc}", name=f"qout{rc}")
                for rc in range(len(ROW_CHUNKS))
            ]
            CH = C // 2  # 4096 pair-scores per group
            for q in range(4):  # group pair (2q, 2q+1)
                cbh = []  # cbh[0]=u-codebook, cbh[1]=v-codebook (each [128, 4096])
                for h in range(2):
                    cbt_h = cbp.tile([128, CH], f32, tag="cbt", name=f"cb{q}_{h}")
                    nc.sync.dma_start(
                        out=cbt_h[:], in_=cbt[q, :, h * CH : (h + 1) * CH]
                    )
                    cbh.append(cbt_h)

                def unit(rc, r0, nr, gg_list, pack):
                    # z = u + |v| over the pair axis; scan z; resolve pair via sign dot
                    z = scp.tile([128, CH], f32, tag="scores", name="z")
                    if pack:
                        nc.gpsimd.memset(z[32:64, :], -1.0e38)
                    for half in range(2):
                        q0 = half * 2048
                        psU = ps2.tile([128, 2048], f32, tag="ps", name="psU")
                        psV = ps2.tile([128, 2048], f32, tag="ps", name="psV")
                        av = avp.tile([128, 2048], f32, tag="av", name="av")
                        for sub in range(4):
                            c0 = q0 + sub * 512
                            for gg in gg_list:
                                tp = (gg * 64, gg * 64) if pack else None
                                p0 = gg * 64 if pack else 0
                                nc.tensor.matmul(
                                    psU[p0 : p0 + nr, sub * 512 : (sub + 1) * 512],
                                    xsT[q][gg * 64 : gg * 64 + 64, r0 : r0 + nr],
                                    cbh[0][gg * 64 : gg * 64 + 64, c0 : c0 + 512],
                                    start=True, stop=True, tile_position=tp,
                                )
                        for sub in range(4):
                            c0 = q0 + sub * 512
                            for gg in gg_list:
                                tp = (gg * 64, gg * 64) if pack else None
                                p0 = gg * 64 if pack else 0
                                nc.tensor.matmul(
                                    psV[p0 : p0 + nr, sub * 512 : (sub + 1) * 512],
                                    xsT[q][gg * 64 : gg * 64 + 64, r0 : r0 + nr],
                                    cbh[1][gg * 64 : gg * 64 + 64, c0 : c0 + 512],
                                    start=True, stop=True, tile_position=tp,
                                )
                        for gg in gg_list:
                            p0 = gg * 64 if pack else 0
                            nc.scalar.activation(
                                out=av[p0 : p0 + nr, :],
                                in_=psV[p0 : p0 + nr, :],
                                func=mybir.ActivationFunctionType.Abs,
                            )
                            nc.vector.tensor_tensor(
                                out=z[p0 : p0 + nr, q0 : q0 + 2048],
                                in0=psU[p0 : p0 + nr, :],
                                in1=av[p0 : p0 + nr, :],
                                op=mybir.AluOpType.add,
                            )
                    np_hi = 64 + nr if pack else nr
                    mx = smp.tile([128, 8], f32, tag="mx", name="mx")
                    ix = smp.tile([128, 8], u32, tag="ix", name="ix")
                    nc.vector.max(out=mx[:np_hi, :], in_=z[:np_hi, :])
                    nc.vector.max_index(
                        out=ix[:np_hi, :], in_max=mx[:np_hi, :], in_values=z[:np_hi, :]
                    )
                    # pair index per row, partition-0 aligned, one column per gg
                    cst = smp.tile([128, 2], u32, tag="cst", name="cst")
                    if pack:
                        nc.vector.tensor_copy(out=cst[:nr, 0:1], in_=ix[:nr, 0:1])
                        nc.sync.dma_start(
                            out=cst[:nr, 1:2], in_=ix[64 : 64 + nr, 0:1]
                        )
                    else:
                        nc.vector.tensor_copy(
                            out=cst[:nr, gg_list[0] : gg_list[0] + 1], in_=ix[:nr, 0:1]
                        )
                    for gg in gg_list:
                        g = 2 * q + gg
                        cbdrow = smp.tile([128, Dg], f32, tag="cbdrow", name="cbdrow")
                        nc.gpsimd.indirect_dma_start(
                            out=cbdrow[:nr, :],
                            out_offset=None,
                            in_=cbd[g][:],
                            in_offset=bass.IndirectOffsetOnAxis(
                                ap=cst[:nr, gg : gg + 1], axis=0
                            ),
                        )
                        junk64 = smp.tile([128, Dg], f32, tag="junk64", name="junk64")
                        vdot = smp.tile([128, 2], f32, tag="vdot", name="vdot")
                        nc.vector.tensor_tensor(
                            out=junk64[:nr, :],
                            in0=cbdrow[:nr, :],
                            in1=xs_nat[rc][:nr, g * Dg : (g + 1) * Dg],
                            op=mybir.AluOpType.mult,
                        )
                        nc.vector.tensor_reduce(
                            out=vdot[:nr, 0:1],
                            in_=junk64[:nr, :],
                            axis=mybir.AxisListType.X,
                            op=mybir.AluOpType.add,
                        )
                        # b = 1 if vdot < 0 else 0 ; code = 2*c' + b
                        nc.vector.tensor_scalar(
                            vdot[:nr, 1:2], vdot[:nr, 0:1], 0.0,
                            scalar2=None, op0=mybir.AluOpType.is_lt,
                        )
                        ixf = smp.tile([128, 1], f32, tag="ixf", name="ixf")
                        nc.vector.tensor_copy(
                            out=ixf[:nr, :], in_=cst[:nr, gg : gg + 1]
                        )
                        codef = smp.tile([128, 1], f32, tag="codef", name="codef")
                        nc.vector.scalar_tensor_tensor(
                            out=codef[:nr, :], in0=ixf[:nr, :], scalar=2.0,
                            in1=vdot[:nr, 1:2],
                            op0=mybir.AluOpType.mult, op1=mybir.AluOpType.add,
                        )
                        codeu = smp.tile([128, 1], u32, tag="codeu", name="codeu")
                        nc.vector.tensor_copy(out=codeu[:nr, :], in_=codef[:nr, :])
                        nc.vector.tensor_copy(
                            out=codes_sb[rc][:nr, g : g + 1], in_=codeu[:nr, :]
                        )
                        nc.gpsimd.indirect_dma_start(
                            out=qsb_all[rc][:nr, g * Dg : (g + 1) * Dg],
                            out_offset=None,
                            in_=cbg[g][:],
                            in_offset=bass.IndirectOffsetOnAxis(
                                ap=codeu[:nr, :], axis=0
                            ),
                        )

                for rc, (r0, nr) in enumerate(ROW_CHUNKS[:2]):
                    for gg in range(2):
                        unit(rc, r0, nr, [gg], pack=False)
                r0, nr = ROW_CHUNKS[2]
                unit(2, r0, nr, [0, 1], pack=True)

            # ---- write outputs ----
            for rc, (r0, nr) in enumerate(ROW_CHUNKS):
                nc.sync.dma_start(out=quant[r0 : r0 + nr, :], in_=qsb_all[rc][:nr, :])
                nc.sync.dma_start(
                    out=codes[r0 : r0 + nr, :], in_=codes_sb[rc][:nr, :]
                )

    nc.compile()
    return nc


def _install_ntff_hook():
    """Shim antenv.axon_hooks so run_bass_kernel_spmd(trace=True) can profile."""
    import types

    try:
        from antenv.axon_hooks import get_axon_ntff_profile_hook  # noqa: F401

        return
    except ImportError:
        pass
    sys.path.insert(0, "/root/.axon_site")
    from trn_agent_boot.trn_boot import _ntff_profile_via_ctypes

    hook = _ntff_profile_via_ctypes("/opt/axon/libaxon_pjrt.so")
    mod = types.ModuleType("antenv.axon_hooks")
    mod.get_axon_ntff_profile_hook = lambda: hook
    mod.set_axon_ntff_profile_hook = lambda h: None
    import antenv

    sys.modules["antenv.axon_hooks"] = mod
    antenv.axon_hooks = mod


def kernel(feats, projection, codebook):
    from concourse.bass_utils import run_bass_kernel_spmd

    if os.environ.get("VQ_TRACE"):
        _install_ntff_hook()
    if "nc" not in _CACHE:
        _CACHE["nc"] = _build()
    nc = _CACHE["nc"]

    feats = np.ascontiguousarray(feats, dtype=np.float32)
    projection = np.ascontiguousarray(projection, dtype=np.float32)
    codebook = np.ascontiguousarray(codebook, dtype=np.float32)

    cbT = codebook.transpose(1, 2, 0)  # [G, Dg, C]
    u = (cbT[:, :, 0::2] + cbT[:, :, 1::2]) * 0.5  # [G, Dg, C/2]
    v = (cbT[:, :, 0::2] - cbT[:, :, 1::2]) * 0.5
    cbt_np = np.ascontiguousarray(
        np.concatenate([u, v], axis=2).reshape(4, 128, C)
    )
    cbd_np = [
        np.ascontiguousarray((codebook[0::2, g, :] - codebook[1::2, g, :]) * 0.5)
        for g in range(G)
    ]
    cbg_np = [np.ascontiguousarray(codebook[:, g, :]) for g in range(G)]

    in_maps = []
    for b in range(NCORES):
        m = {
            "feats": np.ascontiguousarray(feats[b]),
            "proj": projection,
            "cbt": cbt_np,
        }
        for g in range(G):
            m[f"cbg{g}"] = cbg_np[g]
            m[f"cbd{g}"] = cbd_np[g]
        in_maps.append(m)

    trace = bool(os.environ.get("VQ_TRACE"))
    res = run_bass_kernel_spmd(
        nc, in_maps, core_ids=list(range(NCORES)), trace=trace
    )
    _CACHE["profile"] = {
        "exec_time_ns": res.exec_time_ns,
        "instructions_and_trace": res.instructions_and_trace,
    }

    quant = np.stack([res.results[b]["quant"] for b in range(NCORES)])
    codes = np.stack([res.results[b]["codes"] for b in range(NCORES)])
    return quant.reshape(B, T2, D), codes.astype(np.int32).reshape(B, T2, G)


# revision 30
# speedup vs baseline: 1.6979x; 1.6979x over previous
"""BestRQ VQ kernel for 8 TRN2 NeuronCores.

Data-parallel over batch: core b handles feats[b] (299 stacked rows).
codes = argmax_c <x_g, cb_cg>  (L2-norm of x is a positive per-row scalar and
the codebook rows are unit-norm, so normalization does not change the argmin).
quantized = codebook[codes] gathered via indirect DMA.
"""

import os
import sys

import numpy as np

sys.path.insert(0, "/opt/trn_rl_repo")

B, T, M = 8, 1200, 80
F, S = 7, 4
T2 = (T - F) // S + 1  # 299
D, G, Dg, C = 512, 8, 64, 8192
KIN = F * M  # 560
KT = 112  # KIN split into 5 tiles of 112
NKT = 5
NCORES = 8
ROW_CHUNKS = [(0, 128), (128, 128), (256, T2 - 256)]  # (start, nrows)
NCT = C // 512  # 16 c-tiles of 512

_CACHE = {}


def _build():
    import concourse.bass as bass
    import concourse.tile as tile
    from concourse import bacc, mybir
    from concourse.masks import make_identity

    f32 = mybir.dt.float32
    f32r = mybir.dt.float32r
    u32 = mybir.dt.uint32

    nc = bacc.Bacc(None, target_bir_lowering=False)

    feats = nc.declare_dram_parameter("feats", [T, M], f32, isOutput=False)
    proj = nc.declare_dram_parameter("proj", [KIN, D], f32, isOutput=False)
    cbt = nc.declare_dram_parameter("cbt", [4, 128, C], f32, isOutput=False)
    cbg = [
        nc.declare_dram_parameter(f"cbg{g}", [C, Dg], f32, isOutput=False)
        for g in range(G)
    ]
    quant = nc.declare_dram_parameter("quant", [T2, D], f32, isOutput=True)
    codes = nc.declare_dram_parameter("codes", [T2, G], u32, isOutput=True)

    with tile.TileContext(nc) as tc:
        with (
            tc.tile_pool(name="const", bufs=1) as constp,
            tc.tile_pool(name="stage1", bufs=1) as s1p,
            tc.tile_pool(name="cb", bufs=3) as cbp,
            tc.tile_pool(name="scores", bufs=3) as scp,
            tc.tile_pool(name="outs", bufs=1) as outp,
            tc.tile_pool(name="small", bufs=4) as smp,
            tc.tile_pool(name="probe", bufs=1) as probep,
            tc.tile_pool(name="psum2", bufs=2, space="PSUM") as ps2,
        ):
            ident = constp.tile([128, 128], f32)
            make_identity(nc, ident[:])

            # ---- Stage 1: stacked features (strided DMA) + transpose ----
            stT = [s1p.tile([KT, T2], f32, tag=f"stT{k}", name=f"stT{k}") for k in range(NKT)]
            for rc, (r0, nr) in enumerate(ROW_CHUNKS):
                stacked = s1p.tile([128, KIN], f32, tag=f"stacked{rc}", name=f"stacked{rc}")
                src = bass.AP(
                    tensor=feats, offset=r0 * S * M, ap=[[S * M, nr], [1, KIN]]
                )
                nc.sync.dma_start(out=stacked[:nr, :], in_=src)
                for k in range(NKT):
                    pt = ps2.tile([KT, 128], f32, tag="ps", name="tp")
                    nc.tensor.transpose(
                        out=pt[:, :nr],
                        in_=stacked[:nr, k * KT : (k + 1) * KT],
                        identity=ident[:nr, :nr],
                    )
                    nc.vector.tensor_copy(
                        out=stT[k][:, r0 : r0 + nr], in_=pt[:, :nr]
                    )

            # ---- projection weights ----
            projsb = []
            for k in range(NKT):
                pw = s1p.tile([KT, D], f32, tag=f"proj{k}", name=f"projsb{k}")
                nc.sync.dma_start(
                    out=pw[:], in_=proj[k * KT : (k + 1) * KT, :]
                )
                projsb.append(pw)

            # ---- xsT = (stacked @ proj).T  as 4 tiles [128, T2] ----
            xsT = []
            for dc in range(4):
                px = ps2.tile([128, T2], f32, tag="ps", name="px")
                for k in range(NKT):
                    nc.tensor.matmul(
                        px[:],
                        projsb[k][:, dc * 128 : (dc + 1) * 128],
                        stT[k][:],
                        start=(k == 0),
                        stop=(k == NKT - 1),
                    )
                xt = s1p.tile([128, T2], f32, tag=f"xsT{dc}", name=f"xsT{dc}")
                nc.vector.tensor_copy(out=xt[:], in_=px[:])
                xsT.append(xt)

            # ---- Stage 2: distances + argmax ----
            codes_sb = [
                outp.tile([128, G], u32, tag=f"codes{rc}", name=f"codes{rc}")
                for rc in range(len(ROW_CHUNKS))
            ]
            qsb_all = [
                outp.tile([128, D], f32, tag=f"qout{rc}", name=f"qout{rc}")
                for rc in range(len(ROW_CHUNKS))
            ]
            for q in range(4):  # group pair (2q, 2q+1)
                cbh = []
                for h in range(2):
                    cbt_h = cbp.tile([128, C // 2], f32, tag="cbt", name=f"cb{q}_{h}")
                    nc.sync.dma_start(
                        out=cbt_h[:], in_=cbt[q, :, h * (C // 2) : (h + 1) * (C // 2)]
                    )
                    cbh.append(cbt_h)
                # --- full row chunks: one scan unit per (rc, gg), sequential ---
                for rc, (r0, nr) in enumerate(ROW_CHUNKS[:2]):
                    for gg in range(2):
                        g = 2 * q + gg
                        sc = scp.tile([128, C], f32, tag="scores", name="sc")
                        for half in range(4):
                            ps = ps2.tile([128, 2048], f32, tag="ps", name="ps")
                            for sub in range(4):
                                ct = half * 4 + sub
                                nc.tensor.matmul(
                                    ps[:nr, sub * 512 : (sub + 1) * 512],
                                    xsT[q][gg * 64 : gg * 64 + 64, r0 : r0 + nr],
                                    cbh[ct // 8][
                                        gg * 64 : gg * 64 + 64,
                                        (ct % 8) * 512 : (ct % 8 + 1) * 512,
                                    ],
                                    start=True,
                                    stop=True,
                                )
                            nc.scalar.copy(
                                out=sc[:nr, half * 2048 : (half + 1) * 2048],
                                in_=ps[:nr, :],
                            )
                        mx = smp.tile([128, 8], f32, tag="mx", name="mx")
                        ix = smp.tile([128, 8], u32, tag="ix", name="ix")
                        nc.vector.max(out=mx[:nr, :], in_=sc[:nr, :])
                        nc.vector.max_index(
                            out=ix[:nr, :], in_max=mx[:nr, :], in_values=sc[:nr, :]
                        )
                        nc.vector.tensor_copy(
                            out=codes_sb[rc][:nr, g : g + 1], in_=ix[:nr, 0:1]
                        )
                        nc.gpsimd.indirect_dma_start(
                            out=qsb_all[rc][:nr, g * Dg : (g + 1) * Dg],
                            out_offset=None,
                            in_=cbg[g][:],
                            in_offset=bass.IndirectOffsetOnAxis(
                                ap=ix[:nr, 0:1], axis=0
                            ),
                        )
                # --- packed 43-row chunk: both groups in one scan unit ---
                r0, nr = ROW_CHUNKS[2]
                scP = scp.tile([128, C], f32, tag="scores", name="scP")
                nc.gpsimd.memset(scP[32:64, :], -1.0e38)
                for half in range(4):
                    psp_t = ps2.tile([128, 2048], f32, tag="ps", name="psP")
                    for sub in range(4):
                        ct = half * 4 + sub
                        for gg in range(2):
                            nc.tensor.matmul(
                                psp_t[gg * 64 : gg * 64 + nr, sub * 512 : (sub + 1) * 512],
                                xsT[q][gg * 64 : gg * 64 + 64, r0 : r0 + nr],
                                cbh[ct // 8][
                                    gg * 64 : gg * 64 + 64,
                                    (ct % 8) * 512 : (ct % 8 + 1) * 512,
                                ],
                                start=True,
                                stop=True,
                                tile_position=(gg * 64, gg * 64),
                            )
                    nc.scalar.copy(
                        out=scP[:nr, half * 2048 : (half + 1) * 2048],
                        in_=psp_t[:nr, :],
                    )
                    nc.scalar.copy(
                        out=scP[64 : 64 + nr, half * 2048 : (half + 1) * 2048],
                        in_=psp_t[64 : 64 + nr, :],
                    )
                mxP = smp.tile([128, 8], f32, tag="mx", name="mxP")
                ixP = outp.tile([128, 8], u32, tag=f"ixP{q}", name=f"ixP{q}")
                nc.vector.max(out=mxP[: 64 + nr, :], in_=scP[: 64 + nr, :])
                nc.vector.max_index(
                    out=ixP[: 64 + nr, :], in_max=mxP[: 64 + nr, :], in_values=scP[: 64 + nr, :]
                )
                nc.vector.tensor_copy(
                    out=codes_sb[2][:nr, 2 * q : 2 * q + 1], in_=ixP[:nr, 0:1]
                )
                nc.sync.dma_start(
                    out=codes_sb[2][:nr, 2 * q + 1 : 2 * q + 2],
                    in_=ixP[64 : 64 + nr, 0:1],
                )

            # ---- write outputs ----
            for rc, (r0, nr) in enumerate(ROW_CHUNKS[:2]):
                nc.sync.dma_start(out=quant[r0 : r0 + nr, :], in_=qsb_all[rc][:nr, :])
                nc.sync.dma_start(
                    out=codes[r0 : r0 + nr, :], in_=codes_sb[rc][:nr, :]
                )
            # rc2: gather all groups from completed codes, then plain writes
            r0, nr = ROW_CHUNKS[2]
            for g in range(G):
                nc.gpsimd.indirect_dma_start(
                    out=qsb_all[2][:nr, g * Dg : (g + 1) * Dg],
                    out_offset=None,
                    in_=cbg[g][:],
                    in_offset=bass.IndirectOffsetOnAxis(
                        ap=codes_sb[2][:nr, g : g + 1], axis=0
                    ),
                )
            nc.sync.dma_start(out=quant[r0 : r0 + nr, :], in_=qsb_all[2][:nr, :])
            nc.sync.dma_start(out=codes[r0 : r0 + nr, :], in_=codes_sb[2][:nr, :])

    nc.compile()
    return nc


def _install_ntff_hook():
    """Shim antenv.axon_hooks so run_bass_kernel_spmd(trace=True) can profile."""
    import types

    try:
        from antenv.axon_hooks import get_axon_ntff_profile_hook  # noqa: F401

        return
    except ImportError:
        pass
    sys.path.insert(0, "/root/.axon_site")
    from trn_agent_boot.trn_boot import _ntff_profile_via_ctypes

    hook = _ntff_profile_via_ctypes("/opt/axon/libaxon_pjrt.so")
    mod = types.ModuleType("antenv.axon_hooks")
    mod.get_axon_ntff_profile_hook = lambda: hook
    mod.set_axon_ntff_profile_hook = lambda h: None
    import antenv

    sys.modules["antenv.axon_hooks"] = mod
    antenv.axon_hooks = mod


def kernel(feats, projection, codebook):
    from concourse.bass_utils import run_bass_kernel_spmd

    if os.environ.get("VQ_TRACE"):
        _install_ntff_hook()
    if "nc" not in _CACHE:
        _CACHE["nc"] = _build()
    nc = _CACHE["nc"]

    feats = np.ascontiguousarray(feats, dtype=np.float32)
    projection = np.ascontiguousarray(projection, dtype=np.float32)
    codebook = np.ascontiguousarray(codebook, dtype=np.float32)

    cbt_np = np.ascontiguousarray(
        codebook.transpose(1, 2, 0).reshape(4, 128, C)
    )
    cbg_np = [np.ascontiguousarray(codebook[:, g, :]) for g in range(G)]

    in_maps = []
    for b in range(NCORES):
        m = {
            "feats": np.ascontiguousarray(feats[b]),
            "proj": projection,
            "cbt": cbt_np,
        }
        for g in range(G):
            m[f"cbg{g}"] = cbg_np[g]
        in_maps.append(m)

    trace = bool(os.environ.get("VQ_TRACE"))
    res = run_bass_kernel_spmd(
        nc, in_maps, core_ids=list(range(NCORES)), trace=trace
    )
    _CACHE["profile"] = {
        "exec_time_ns": res.exec_time_ns,
        "instructions_and_trace": res.instructions_and_trace,
    }

    quant = np.stack([res.results[b]["quant"] for b in range(NCORES)])
    codes = np.stack([res.results[b]["codes"] for b in range(NCORES)])
    return quant.reshape(B, T2, D), codes.astype(np.int32).reshape(B, T2, G)
